# revision 1
# baseline (speedup 1.0000x reference)
"""DGL capsule routing layer on 8 trn2 NeuronCores (Bass/Tile).

Math: for routing_num iterations,
    c = softmax(b, axis=out)                        # b0 = 0
    s = einsum('io,iof->of', c, uh)
    v = squash(s)
    b = b + einsum('iof,of->io', uh, v)
Output: final v [OUT, F].

Key identity: b_t = uh . (v_1 + ... + v_t)  (b is linear in uh), so b is
never materialized across iterations; each iteration is one streaming pass
over uh with w_t = cumulative sum of v's:
    pass t: b = sum_f uh[i,o,f]*w[o,f]; e = exp(b); r_i = 1/sum_o e
            s[o,f] = sum_i r_i * e[i,o] * uh[i,o,f]   (partial per core)
            AllReduce(s); v = squash(s); w += v
Pass 1 has c uniform (=1/OUT) so it is a pure PE pass.

Sharding: i (in_nodes) split across 8 cores, 512 rows each (4 blocks of
128 partitions). Engine plan per 2048-wide o-f chunk (passes >= 2):
  GpSimd: tm = uh * w_bcast        (2-input mul; DVE TT never contends)
  DVE:    b-slice = segsum_f(tm);  p = e * uh (e broadcast over f)
  ACT:    e = exp(b) with fused denominator accum; psum flushes
  PE:     s-partial = sum_i rinv[i]*p[i,:] as 4x N=512 matmuls with
          rinv as the 1-column stationary operand -> psum [1,2048]
The per-block s partials go straight to DRAM [4,16384]; the AllReduce sums
over cores, and the cheap cross-block sum happens after the AR in the
partition-spread [128,128] layout (3 DVE adds).
"""

import numpy as np
from contextlib import ExitStack

import concourse.bass as bass
import concourse.mybir as mybir
import concourse.tile as tile
from concourse import bacc
from concourse import bass_utils

F32 = mybir.dt.float32
AX = mybir.AxisListType
AF = mybir.ActivationFunctionType

IN_NODES, OUT_NODES, F_SIZE = 4096, 1024, 16
CORES = 8
I_LOC = IN_NODES // CORES          # 512 in-nodes per core
ROW = OUT_NODES * F_SIZE           # 16384 floats per in-node row
P = 128
NBLK = I_LOC // P                  # 4 i-blocks per core
QT = 4096                          # streamed quarter width (elems/partition)
NQT = ROW // QT                    # 4 quarters per block
CH = 2048                          # chunk/piece width (elems/partition)
NCH_Q = QT // CH                   # 2 chunks per quarter
NMM = CH // 512                    # 4 matmuls per piece
F32R_MM = True                     # fast-path fp32 matmuls (1 cyc/row)
MM_DT = mybir.dt.float32r if F32R_MM else F32


def _body(nc, tc, uh, v_out, R, rg):
    uh_t = uh.rearrange("(n p) r -> n p r", p=P)   # [NBLK, 128, 16384]

    with ExitStack() as ctx:
        io = ctx.enter_context(tc.tile_pool(name="io", bufs=4))
        work = ctx.enter_context(tc.tile_pool(name="work", bufs=4))
        small = ctx.enter_context(tc.tile_pool(name="small", bufs=2))
        persist = ctx.enter_context(tc.tile_pool(name="persist", bufs=1))
        pspool = ctx.enter_context(tc.tile_pool(name="pspool", bufs=2, space="PSUM"))
        dram = ctx.enter_context(tc.tile_pool(name="dram", bufs=2, space="DRAM"))

        c0_f = persist.tile([P, 1], F32, name="c0_f")
        nc.vector.memset(c0_f, 1.0 / OUT_NODES)
        c0 = persist.tile([P, 1], MM_DT, name="c0")
        nc.vector.tensor_copy(c0, c0_f)
        w_sb = w_acc = None
        if R > 1:
            w_sb = persist.tile([P, ROW], F32, name="w_sb")
            w_acc = persist.tile([P, P], F32, name="w_acc")

        for t in range(1, R + 1):
            ar_in = dram.tile([NBLK, ROW], F32, tag="ar_in")
            for blk in range(NBLK):
                uts = []
                for q in range(NQT):
                    ut = io.tile([P, QT], F32, tag="ut")
                    nc.sync.dma_start(ut, uh_t[blk, :, q * QT:(q + 1) * QT])
                    uts.append(ut)
                if t == 1:
                    rinv = c0
                else:
                    b = small.tile([P, OUT_NODES], F32, tag="b")
                    for q in range(NQT):
                        for k in range(NCH_Q):
                            sl = slice(k * CH, (k + 1) * CH)
                            g0 = q * QT + k * CH
                            tm = work.tile([P, CH], F32, tag="tm")
                            # b-mul on GpSimd (concurrent with DVE TT/reduce)
                            nc.gpsimd.tensor_mul(
                                tm, uts[q][:, sl], w_sb[:, g0:g0 + CH])
                            o0 = g0 // F_SIZE
                            nc.vector.reduce_sum(
                                b[:, o0:o0 + CH // F_SIZE],
                                tm.rearrange("p (o f) -> p o f", f=F_SIZE),
                                axis=AX.X,
                            )
                    e = small.tile([P, OUT_NODES], F32, tag="e")
                    den = small.tile([P, 1], F32, tag="den")
                    nc.scalar.activation(e, b, AF.Exp, accum_out=den)
                    rinv_f = small.tile([P, 1], F32, tag="rinv_f")
                    nc.vector.reciprocal(rinv_f, den)
                    rinv = small.tile([P, 1], MM_DT, tag="rinv")
                    nc.vector.tensor_copy(rinv, rinv_f)
                for q in range(NQT):
                    for k in range(NCH_Q):
                        sl = slice(k * CH, (k + 1) * CH)
                        g0 = q * QT + k * CH
                        pt = work.tile([P, CH], MM_DT, tag="tm")
                        if t == 1:
                            # round to f32r on idle DVE (pass 1 only)
                            nc.vector.tensor_copy(pt, uts[q][:, sl])
                        else:
                            o0 = g0 // F_SIZE
                            och = CH // F_SIZE
                            nc.vector.tensor_mul(
                                pt.rearrange("p (o f) -> p o f", f=F_SIZE),
                                uts[q][:, sl].rearrange(
                                    "p (o f) -> p o f", f=F_SIZE),
                                e[:, o0:o0 + och][:, :, None].broadcast_to(
                                    [P, och, F_SIZE]),
                            )
                        ps = pspool.tile([1, CH], F32, tag="ps")
                        for c in range(NMM):
                            nc.tensor.matmul(
                                ps[:, c * 512:(c + 1) * 512],
                                rinv,
                                pt[:, c * 512:(c + 1) * 512],
                                start=True, stop=True,
                                skip_group_check=True,
                            )
                        fl = small.tile([1, CH], F32, tag="fl")
                        nc.scalar.copy(fl, ps)
                        nc.sync.dma_start(ar_in[blk, g0:g0 + CH], fl)
            ar_out = dram.tile([NBLK, ROW], F32, tag="ar_out")
            nc.gpsimd.collective_compute(
                "AllReduce", mybir.AluOpType.add, replica_groups=rg,
                ins=[ar_in.opt()], outs=[ar_out.opt()],
            )
            # s2[p,(j,f)] with o = p*8+j: sum the 4 block rows post-AR
            slds = []
            for blk in range(NBLK):
                sld = small.tile([P, P], F32, tag="sld", bufs=4)
                nc.sync.dma_start(
                    sld, ar_out[blk].rearrange("(p q) -> p q", p=P))
                slds.append(sld)
            s2 = small.tile([P, P], F32, tag="s2")
            nc.vector.tensor_add(s2, slds[0], slds[1])
            nc.vector.tensor_add(s2, s2, slds[2])
            nc.vector.tensor_add(s2, s2, slds[3])
            # squash: v = s * sqrt(sq)/(1+sq), sq = sum_f s^2
            ssq = small.tile([P, P], F32, tag="ssq")
            nc.vector.tensor_mul(ssq, s2, s2)
            sq = small.tile([P, 8], F32, tag="sq")
            nc.vector.reduce_sum(
                sq, ssq.rearrange("p (j f) -> p j f", f=F_SIZE), axis=AX.X)
            # sqrt via exp(0.5*ln(x)): stays in the exp/ln ACT table set
            lnq = small.tile([P, 8], F32, tag="lnq")
            nc.scalar.activation(lnq, sq, AF.Ln)
            y = small.tile([P, 8], F32, tag="y")
            nc.scalar.activation(y, lnq, AF.Exp, scale=0.5)
            # one Newton step: y <- 0.5*(y + sq/y)
            ry = small.tile([P, 8], F32, tag="ry")
            nc.vector.reciprocal(ry, y)
            t1 = small.tile([P, 8], F32, tag="t1")
            nc.vector.tensor_mul(t1, sq, ry)
            nc.vector.tensor_add(t1, t1, y)
            nc.vector.tensor_scalar_mul(t1, t1, 0.5)
            d1 = small.tile([P, 8], F32, tag="d1")
            nc.vector.tensor_scalar_add(d1, sq, 1.0)
            rd = small.tile([P, 8], F32, tag="rd")
            nc.vector.reciprocal(rd, d1)
            sc = small.tile([P, 8], F32, tag="sc")
            nc.vector.tensor_mul(sc, t1, rd)
            v_sb = small.tile([P, P], F32, tag="v_sb")
            nc.vector.tensor_mul(
                v_sb.rearrange("p (j f) -> p j f", f=F_SIZE),
                s2.rearrange("p (j f) -> p j f", f=F_SIZE),
                sc[:, :, None].broadcast_to([P, 8, F_SIZE]),
            )
            if t == R:
                nc.sync.dma_start(
                    v_out.rearrange("(p j) f -> p (j f)", j=8), v_sb)
            else:
                if t == 1:
                    nc.scalar.copy(w_acc, v_sb)
                else:
                    nc.vector.tensor_add(w_acc, w_acc, v_sb)
                # broadcast w to all partitions via DRAM round-trip:
                # w_acc[p,(j,f)] -> flat w_dram[o*16+f] -> [128, ROW] bcast
                w_dram = dram.tile([ROW], F32, tag="w_dram")
                nc.sync.dma_start(
                    w_dram.rearrange("(p q) -> p q", p=P), w_acc)
                wd_b = w_dram.unsqueeze(0)
                for j in range(8):
                    sl = slice(j * CH, (j + 1) * CH)
                    nc.sync.dma_start(
                        w_sb[:, sl],
                        wd_b[:, sl].broadcast_to([P, CH]))


def _build(routing_num: int):
    R = int(routing_num)
    assert R >= 1
    nc = bacc.Bacc(
        "TRN2", target_bir_lowering=False, debug=False, num_devices=CORES)
    uh = nc.dram_tensor("uh", [I_LOC, ROW], F32, kind="ExternalInput")
    v_out = nc.dram_tensor("v_out", [OUT_NODES, F_SIZE], F32,
                           kind="ExternalOutput")
    rg = [list(range(CORES))]
    with tile.TileContext(nc) as tc:
        _body(nc, tc, uh.ap(), v_out.ap(), R, rg)
    nc.compile()
    return nc


_CACHE: dict = {}


def _get_nc(routing_num: int):
    R = int(routing_num)
    if R not in _CACHE:
        _CACHE[R] = _build(R)
    return _CACHE[R]


def _shard(u_hat: np.ndarray):
    uh = np.ascontiguousarray(np.asarray(u_hat, dtype=np.float32))
    assert uh.shape == (IN_NODES * OUT_NODES, F_SIZE), uh.shape
    uh = uh.reshape(IN_NODES, ROW)
    return [
        {"uh": np.ascontiguousarray(uh[k * I_LOC:(k + 1) * I_LOC])}
        for k in range(CORES)
    ]


def run(u_hat, routing_num, trace=False):
    nc = _get_nc(routing_num)
    in_maps = _shard(u_hat)
    res = bass_utils.run_bass_kernel_spmd(
        nc, in_maps, core_ids=list(range(CORES)), trace=trace)
    return res


def kernel(u_hat, routing_num):
    res = run(u_hat, routing_num, trace=False)
    return np.asarray(res.results[0]["v_out"], dtype=np.float32)



# revision 6
# speedup vs baseline: 1.3662x; 1.3662x over previous
"""DGL capsule routing layer on 8 trn2 NeuronCores (Bass/Tile) — v2.

Math per iteration (b0 = 0):
    c = softmax(b, axis=out); s = einsum('io,iof->of', c, uh)
    v = squash(s); b += einsum('iof,of->io', uh, v)
Output: final v [OUT, F].

Key identity: b_t = uh . w_{t-1} with w = cumulative sum of v's, so b is
recomputed each pass from w instead of being carried.

v2 design (vs v1 which re-streamed uh from HBM every pass and used GpSimd
for the big muls):
  * uh is DMA'd from HBM ONCE (pass 1), converted f32->bf16 and cached in
    SBUF in an f-OUTER layout [p=i, (f,o)] (flat col g = f*1024 + o).
    bf16 keeps DVE tensor_tensor in the 2x_1p fast mode and PE matmuls at
    1 cyc/row; the all-bf16 pipeline sims at ~3e-3 rel err vs 2e-2 budget.
  * Pass t>=2, per 128-i block: tm = uh*w (2 TT muls over halves), b =
    in-place halving-tree sum of tm over f, e = exp(b) on ACT with fused
    denominator accum, rinv = 1/den (bf16, folded into the PE stationary).
  * s partials: per f-plane, pt = e*uh (TT mul) then PE matmul with rinv
    as 1-col stationary, PSUM-accumulating over the 4 i-blocks, so the AR
    payload is [16384] bf16 once (32 KiB vs v1's 256 KiB f32).
  * Post-AR squash entirely on-chip in a p-major [128,128] layout
    (partition p = f*8+o_hi, free q = o_lo); the cross-partition f-sum and
    the sc broadcast are one-hot PE matmuls (oh1: p%8==m, oh2: m%8==p).
  * w accumulates in bf16 [128,128]; broadcast to all partitions goes
    through a [16384] bf16 DRAM bounce (p-major flat == f-major flat) read
    back with a partition-stride-0 broadcast into w_fo [p, (f,o)].
  * Output v is bf16 [128,128] p-major; the host wrapper unpacks to
    [1024, 16] f32.
SBUF is packed to ~206 KiB/partition: pass-1 f32 staging, the tm tree
buffers, and the pt tiles all share one 2x16KiB pool tag; the b tile and
the 16 AR-flush tiles share another.
"""

import numpy as np
from contextlib import ExitStack

import concourse.bass as bass
import concourse.mybir as mybir
import concourse.tile as tile
from concourse import bacc
from concourse import bass_utils

F32 = mybir.dt.float32
BF16 = mybir.dt.bfloat16
AF = mybir.ActivationFunctionType
AO = mybir.AluOpType

IN_NODES, OUT_NODES, F_SIZE = 4096, 1024, 16
CORES = 8
I_LOC = IN_NODES // CORES          # 512 in-nodes per core
ROW = OUT_NODES * F_SIZE           # 16384 values per in-node row
P = 128
NBLK = I_LOC // P                  # 4 i-blocks per core
QT = 4096                          # pass-1 staging chunk (f32 elems/partition)
NQT = ROW // QT                    # 4 chunks per block
O = OUT_NODES
H = ROW // 2                       # 8192


def _body(nc, tc, uh, v_out, R, rg):
    uh_t = uh.rearrange("(n p) r -> n p r", p=P)   # [NBLK, 128, 16384] f32

    with ExitStack() as ctx:
        persist = ctx.enter_context(tc.tile_pool(name="persist", bufs=1))
        scp = ctx.enter_context(tc.tile_pool(name="scp", bufs=2))
        smp = ctx.enter_context(tc.tile_pool(name="smp", bufs=1))
        psp = ctx.enter_context(tc.tile_pool(name="psp", bufs=3, space="PSUM"))
        dram = ctx.enter_context(tc.tile_pool(name="dram", bufs=2, space="DRAM"))

        # --- persistent tiles -------------------------------------------
        uhb = [persist.tile([P, ROW], BF16, name=f"uhb{k}", tag=f"uhb{k}")
               for k in range(NBLK)]
        w_fo = None
        if R > 1:
            w_fo = persist.tile([P, ROW], BF16, name="w_fo")
        c0 = persist.tile([P, 1], BF16, name="c0")
        nc.vector.memset(c0, 1.0 / OUT_NODES)
        # one-hot stationaries for the squash cross-partition ops
        oh1_d = nc.inline_tensor(
            (np.arange(P)[:, None] % 8 == np.arange(8)[None, :])
            .astype(np.float32), name="oh1d")
        oh2_d = nc.inline_tensor(
            (np.arange(P)[None, :] % 8 == np.arange(8)[:, None])
            .astype(np.float32), name="oh2d")
        oh1 = persist.tile([P, 8], F32, name="oh1")
        nc.sync.dma_start(oh1, oh1_d.ap())
        oh2 = persist.tile([8, P], F32, name="oh2")
        nc.sync.dma_start(oh2, oh2_d.ap())

        w_acc_prev = None

        for t in range(1, R + 1):
            ar_in = dram.tile([ROW], BF16, tag="ar_in")
            if t == 1:
                # ---- pass 1: stream uh, convert to bf16 f-outer cache ----
                for blk in range(NBLK):
                    for q in range(NQT):
                        st = scp.tile([P, QT], F32, tag="sc", name="st")
                        nc.sync.dma_start(
                            st, uh_t[blk, :, q * QT:(q + 1) * QT])
                        # (o,f) f-inner chunk -> f-outer cache columns
                        dst = uhb[blk].rearrange("p (f o) -> p f o", o=O)[
                            :, :, q * (QT // F_SIZE):(q + 1) * (QT // F_SIZE)]
                        nc.vector.tensor_copy(
                            dst, st.rearrange("p (o f) -> p f o", f=F_SIZE))
                rbs = [c0] * NBLK
                e2s = None
            else:
                # ---- passes >= 2: b, e, rinv per block from SBUF cache ----
                rbs, e2s = [], []
                for blk in range(NBLK):
                    tmA = scp.tile([P, H], BF16, tag="sc", name="tmA")
                    tmB = scp.tile([P, H], BF16, tag="sc", name="tmB")
                    nc.vector.tensor_mul(tmA, uhb[blk][:, 0:H], w_fo[:, 0:H])
                    nc.vector.tensor_mul(
                        tmB, uhb[blk][:, H:ROW], w_fo[:, H:ROW])
                    # in-place halving tree: sum over the 16 f-planes
                    nc.vector.tensor_add(tmA, tmA, tmB)
                    nc.vector.tensor_add(
                        tmA[:, 0:4096], tmA[:, 0:4096], tmA[:, 4096:8192])
                    nc.vector.tensor_add(
                        tmA[:, 0:2048], tmA[:, 0:2048], tmA[:, 2048:4096])
                    b = smp.tile([P, O], BF16, tag="bfl", name="b")
                    nc.vector.tensor_add(
                        b, tmA[:, 0:1024], tmA[:, 1024:2048])
                    e2 = smp.tile([P, O], BF16, tag=f"e2_{blk}", name="e2")
                    den = smp.tile([P, 1], F32, tag="den", name="den")
                    nc.scalar.activation(e2, b, AF.Exp, accum_out=den)
                    rinv = smp.tile([P, 1], F32, tag="rinv", name="rinv")
                    nc.vector.reciprocal(rinv, den)
                    rb = smp.tile([P, 1], BF16, tag=f"rb_{blk}", name="rb")
                    nc.vector.tensor_copy(rb, rinv)
                    rbs.append(rb)
                    e2s.append(e2)

            # ---- s partials: per f-plane, PSUM-accumulate over blocks ----
            for f in range(F_SIZE):
                ps = psp.tile([1, O], F32, tag="ps", name="ps")
                for blk in range(NBLK):
                    if t == 1:
                        mv = uhb[blk][:, f * O:(f + 1) * O]
                    else:
                        pt = scp.tile([P, O], BF16, tag="sc", name="pt")
                        nc.vector.tensor_mul(
                            pt, uhb[blk][:, f * O:(f + 1) * O], e2s[blk])
                        mv = pt
                    for w0 in (0, 512):
                        nc.tensor.matmul(
                            ps[:, w0:w0 + 512], rbs[blk], mv[:, w0:w0 + 512],
                            start=(blk == 0), stop=(blk == NBLK - 1),
                            skip_group_check=True)
                fl = smp.tile([1, O], BF16, tag="bfl", name="fl")
                nc.scalar.copy(fl, ps)
                nc.sync.dma_start(ar_in[f * O:(f + 1) * O], fl)

            ar_out = dram.tile([ROW], BF16, tag="ar_out")
            nc.gpsimd.collective_compute(
                "AllReduce", AO.add, replica_groups=rg,
                ins=[ar_in.opt()], outs=[ar_out.opt()],
            )

            # ---- squash in p-major layout: p = f*8+o_hi, q = o_lo ----
            sld = smp.tile([P, P], BF16, tag="sld", name="sld")
            nc.sync.dma_start(sld, ar_out.rearrange("(p q) -> p q", p=P))
            ssq = smp.tile([P, P], F32, tag="ssq", name="ssq")
            nc.vector.tensor_mul(ssq, sld, sld)
            sqps = psp.tile([8, P], F32, tag="sqps", bufs=1, name="sqps")
            nc.tensor.matmul(sqps, oh1, ssq, start=True, stop=True,
                             skip_group_check=True)
            sq = smp.tile([8, P], F32, tag="sq", name="sq")
            nc.scalar.copy(sq, sqps)
            # sqrt(sq) via exp(0.5*ln) + one Newton step (exp/ln table set)
            lnq = smp.tile([8, P], F32, tag="lnq", name="lnq")
            nc.scalar.activation(lnq, sq, AF.Ln)
            y = smp.tile([8, P], F32, tag="y", name="y")
            nc.scalar.activation(y, lnq, AF.Exp, scale=0.5)
            ry = smp.tile([8, P], F32, tag="ry", name="ry")
            nc.vector.reciprocal(ry, y)
            t1 = smp.tile([8, P], F32, tag="t1", name="t1")
            nc.vector.tensor_mul(t1, sq, ry)
            nc.vector.tensor_add(t1, t1, y)        # t1 = sq/y + y = 2*sqrt
            d2 = smp.tile([8, P], F32, tag="lnq", name="d2")
            nc.vector.tensor_scalar(d2, sq, 1.0, 2.0, AO.add, AO.mult)
            rd = smp.tile([8, P], F32, tag="ry", name="rd")
            nc.vector.reciprocal(rd, d2)           # rd = 0.5/(1+sq)
            sc = smp.tile([8, P], F32, tag="sq", name="sc")
            nc.vector.tensor_mul(sc, t1, rd)       # sqrt(sq)/(1+sq)
            srps = psp.tile([P, P], F32, tag="srps", bufs=1, name="srps")
            nc.tensor.matmul(srps, oh2, sc, start=True, stop=True,
                             skip_group_check=True)
            v_sb = smp.tile([P, P], BF16, tag="v_sb", name="v_sb")
            nc.vector.tensor_mul(v_sb, sld, srps)

            if t == R:
                nc.sync.dma_start(v_out, v_sb)
            else:
                w_acc = smp.tile([P, P], BF16, tag="w_acc", bufs=2,
                                 name="w_acc")
                if t == 1:
                    nc.scalar.copy(w_acc, v_sb)
                else:
                    nc.vector.tensor_add(w_acc, w_acc_prev, v_sb)
                w_acc_prev = w_acc
                w_dram = dram.tile([ROW], BF16, tag="w_dram")
                nc.sync.dma_start(
                    w_dram.rearrange("(p q) -> p q", p=P), w_acc)
                wd_b = w_dram.unsqueeze(0)
                for h in (0, H):
                    nc.sync.dma_start(
                        w_fo[:, h:h + H],
                        wd_b[:, h:h + H].broadcast_to([P, H]))


def _build(routing_num: int):
    R = int(routing_num)
    assert R >= 1
    nc = bacc.Bacc(
        "TRN2", target_bir_lowering=False, debug=False, num_devices=CORES)
    uh = nc.dram_tensor("uh", [I_LOC, ROW], F32, kind="ExternalInput")
    v_out = nc.dram_tensor("v_out", [P, P], BF16, kind="ExternalOutput")
    rg = [list(range(CORES))]
    with tile.TileContext(nc) as tc:
        _body(nc, tc, uh.ap(), v_out.ap(), R, rg)
    nc.compile()
    return nc


_CACHE: dict = {}


def _get_nc(routing_num: int):
    R = int(routing_num)
    if R not in _CACHE:
        _CACHE[R] = _build(R)
    return _CACHE[R]


def _shard(u_hat: np.ndarray):
    uh = np.ascontiguousarray(np.asarray(u_hat, dtype=np.float32))
    assert uh.shape == (IN_NODES * OUT_NODES, F_SIZE), uh.shape
    uh = uh.reshape(IN_NODES, ROW)
    return [
        {"uh": np.ascontiguousarray(uh[k * I_LOC:(k + 1) * I_LOC])}
        for k in range(CORES)
    ]


def run(u_hat, routing_num, trace=False):
    nc = _get_nc(routing_num)
    in_maps = _shard(u_hat)
    res = bass_utils.run_bass_kernel_spmd(
        nc, in_maps, core_ids=list(range(CORES)), trace=trace)
    return res


def _unpack(v_pm) -> np.ndarray:
    # [128,128] p-major bf16, p = f*8+o_hi, q = o_lo  ->  [1024, 16] f32
    v = np.asarray(v_pm).astype(np.float32).reshape(F_SIZE, 8, P)
    return np.ascontiguousarray(v.transpose(1, 2, 0).reshape(OUT_NODES, F_SIZE))


def kernel(u_hat, routing_num):
    res = run(u_hat, routing_num, trace=False)
    return _unpack(res.results[0]["v_out"])
